# revision 12
# baseline (speedup 1.0000x reference)
"""Trainium2 Bass kernel for CannyExtractor (NMS-suppressed canny magnitude).

Contract: kernel(x) takes FULL input x [16,3,512,512] f32, returns FULL output
[16,3,512,512] f32. Internally: batch sharded over 8 NeuronCores (2 images
per core), one SPMD Bass program, device emits the fp16 single-channel
suppressed magnitude; host widens to f32 and replicates the 3 identical
channels (reference output is channel-replicated).

Pipeline per image (fp32 until squares — fp16 compare field; the precision
split is forced: quantizing anything upstream of gx/gy to fp16 pushes L2
rel-err past the 2e-2 gate due to cancellation in the derivative taps):
  gray (DVE STT fp32) -> horizontal 5-tap gaussian (POOL adds + DVE STTs,
  fp32) -> vertical composite convs on PE (banded fp32 matmuls + corner
  matmul for inter-block halos, direct PSUM->SBUF flat relays on ACT) ->
  3-tap horizontal gradients (DVE/POOL fp32) -> squares on ACT (fp32) ->
  s16 fp16 compare field; NMS masks as strict-only compares (is_lt/is_gt;
  is_ge measured 3.2x slower than is_gt on DVE) -> row-shifted planes via
  fp16 PE matmuls -> pair maxes + copy_predicated select chain -> keep,
  sqrt (ACT, +eps bias), clip, apply -> fp16 out, one DMA per image.
"""
import sys
import numpy as np

sys.path.insert(0, "/opt/trn_rl_repo")

H = W = 512
NT = 4            # 128-row blocks per image
P = 128
PAD = 2
WP = W + 2 * PAD  # padded plane width (516)
L = NT * WP       # flat free length (2064)
LV = L - 4        # flat op length (2060)
INT = slice(PAD, PAD + W)
NI = 2            # images per core
NCORES = 8

GRAY = np.array([0.299, 0.587, 0.114], np.float32)
SQT2 = np.float32(np.sqrt(2.0) - 1.0)        # tan(22.5 deg)
T2 = float(np.float32(SQT2 * SQT2))          # tan^2(22.5)
TH2 = float(np.float32(1.0 / (SQT2 * SQT2)))  # tan^2(67.5)


def _gauss5():
    ax = np.arange(5, dtype=np.float32) - 2.0
    g1 = np.exp(-0.5 * ax * ax).astype(np.float32)
    return (g1 / g1.sum()).astype(np.float32)


def _vert_matrix(kind):
    g1 = _gauss5()
    I = np.eye(H, dtype=np.float64)
    X = np.pad(I, ((2, 2), (0, 0)), mode="reflect")
    B = np.zeros((H, H))
    for k in range(5):
        B += g1[k] * X[k:k + H]
    Y = np.pad(B, ((1, 1), (0, 0)), mode="edge")
    taps = [1.0, 2.0, 1.0] if kind == "smooth" else [-1.0, 0.0, 1.0]
    M = np.zeros((H, H))
    for k in range(3):
        if taps[k] != 0.0:
            M += taps[k] * Y[k:k + H]
    return M


def _build_consts():
    Ms = (_vert_matrix("smooth") * float(GRAY[2])).astype(np.float32)
    Md = (_vert_matrix("diff") * float(GRAY[2])).astype(np.float32)
    vs = np.zeros((P, NT, P), np.float32)
    vd = np.zeros((P, NT, P), np.float32)
    for t in range(NT):
        vs[:, t, :] = Ms[128 * t:128 * (t + 1), 128 * t:128 * (t + 1)].T
        vd[:, t, :] = Md[128 * t:128 * (t + 1), 128 * t:128 * (t + 1)].T
    vcor = np.zeros((36, 2, 18), np.float32)
    for b in range(3):
        in_rows = [128 * b + 122 + k for k in range(12)]
        out_rows = [128 * b + 125, 128 * b + 126, 128 * b + 127,
                    128 * (b + 1), 128 * (b + 1) + 1, 128 * (b + 1) + 2]
        for k, ir in enumerate(in_rows):
            for m, orr in enumerate(out_rows):
                vcor[12 * b + k, 0, 6 * b + m] = Ms[orr, ir]
                vcor[12 * b + k, 1, 6 * b + m] = Md[orr, ir]
    # fp16 row-shift matrices: up[k,m]=1 iff k=m+1 (U[m]=s[m+1]); dn: k=m-1
    shm = np.zeros((P, 2, P), np.float16)
    for m in range(P - 1):
        shm[m + 1, 0, m] = 1.0
    for m in range(1, P):
        shm[m - 1, 1, m] = 1.0
    return {"vs": vs, "vd": vd, "vcor": vcor, "shm16": shm}


_CACHE = {}


def _emit_image(nc, tc, pools, tens, img):
    """Generator: yields between stages so the caller interleaves 2 images."""
    import concourse.mybir as mybir
    AL = mybir.AluOpType
    AF = mybir.ActivationFunctionType
    F32 = mybir.dt.float32
    F16 = mybir.dt.float16
    U16 = mybir.dt.uint16

    pwork, pw16, psmall, (ppsumv, ppsumc) = pools
    (xdram, ydram, c_vs, c_vd, c_vcor, c_shm16, zrow16) = tens

    g1 = _gauss5()
    C0, C1, C2 = float(g1[2]), float(g1[1]), float(g1[0])
    R01 = float(np.float32(GRAY[0] / GRAY[1]))
    R12 = float(np.float32(GRAY[1] / GRAY[2]))

    def wt(tag):
        t = pwork.tile([P, NT, WP], F32, tag=tag, name=tag)
        return t, t[:].rearrange("p t w -> p (t w)")

    def wt16(tag, dt=F16):
        t = pw16.tile([P, NT, WP], dt, tag=tag, name=tag)
        return t, t[:].rearrange("p t w -> p (t w)")

    # ---- load input channels (1 DMA per channel, 3D AP) ----
    xa, xaf = wt("A")
    xb, xbf = wt("B")
    xc, xcf = wt("C")
    for h in range(2):
        for c, t in enumerate((xa, xb, xc)):
            nc.sync.dma_start(
                t[:, 2 * h:2 * h + 2, INT],
                xdram[img, c].rearrange("(t p) w -> p t w", p=P)[:, 2 * h:2 * h + 2, :])
    yield

    # ---- grayscale (fp32; scale 1/GRAY[2], folded into vertical mats) ----
    nc.vector.scalar_tensor_tensor(xbf[:, PAD:L], xaf[:, PAD:L],
                                   R01, xbf[:, PAD:L], AL.mult, AL.add)
    nc.vector.scalar_tensor_tensor(xcf[:, PAD:L], xbf[:, PAD:L],
                                   R12, xcf[:, PAD:L], AL.mult, AL.add)
    g, gf = xc, xcf          # gray plane, base-2
    nc.scalar.copy(g[:, :, 1:2], g[:, :, 3:4])
    nc.scalar.copy(g[:, :, 0:1], g[:, :, 4:5])
    nc.scalar.copy(g[:, :, WP - 2:WP - 1], g[:, :, WP - 4:WP - 3])
    nc.scalar.copy(g[:, :, WP - 1:WP], g[:, :, WP - 5:WP - 4])
    yield

    # ---- horizontal 5-tap gaussian (fp32), gh base-2 = blur/C0 ----
    a1, a1f = xa, xaf        # xa dead after gray STT1
    a2, a2f = xb, xbf        # xb dead after gray STT2
    nc.gpsimd.tensor_tensor(a1f[:, 0:LV], gf[:, 1:1 + LV], gf[:, 3:3 + LV],
                            AL.add)
    nc.gpsimd.tensor_tensor(a2f[:, 0:LV], gf[:, 0:LV], gf[:, 4:4 + LV],
                            AL.add)
    nc.vector.scalar_tensor_tensor(a1f[:, 0:LV], a2f[:, 0:LV], C2 / C1,
                                   a1f[:, 0:LV], AL.mult, AL.add)
    gh, ghf = wt("D")
    nc.vector.scalar_tensor_tensor(ghf[:, 2:2 + LV], a1f[:, 0:LV], C1 / C0,
                                   gf[:, 2:2 + LV], AL.mult, AL.add)
    yield

    # ---- vertical composite convs on PE (fp32) ----
    # corner-halo chain first so it overlaps the main banded matmuls
    u1, u1f = xa, xaf        # a1 dead after gh
    u2, u2f = xb, xbf        # a2 dead after gh
    cs = psmall.tile([36, W], F32, tag="cs", name="cs")
    for b in range(3):
        nc.gpsimd.dma_start(cs[12 * b:12 * b + 6, :], gh[122:128, b, INT])
        nc.gpsimd.dma_start(cs[12 * b + 6:12 * b + 12, :], gh[0:6, b + 1, INT])
    cos = []
    for ci in (0, 1):
        cps = ppsumc.tile([18, W], F32, tag="cps", name="cps")
        nc.tensor.matmul(cps[:], c_vcor[:, ci, :], cs[:], start=True, stop=True)
        co = psmall.tile([18, W], F32, tag="co", name="co")
        nc.scalar.copy(co[:], cps[:])
        cos.append(co)
    for (cm, u) in ((c_vs, u1), (c_vd, u2)):
        for h in range(2):
            ps = ppsumv.tile([P, 2, W], F32, tag="psv", name="psv")
            for k in range(2):
                t = 2 * h + k
                nc.tensor.matmul(ps[:, k, :], cm[:, t, :], gh[:, t, INT],
                                 start=True, stop=True)
            nc.scalar.copy(u[:, 2 * h:2 * h + 2, INT], ps[:])
    for ci, u in ((0, u1), (1, u2)):
        co = cos[ci]
        for b in range(3):
            nc.gpsimd.dma_start(u[125:128, b, INT], co[6 * b:6 * b + 3, :])
            nc.gpsimd.dma_start(u[0:3, b + 1, INT], co[6 * b + 3:6 * b + 6, :])
    for u in (u1, u2):
        nc.scalar.copy(u[:, :, 1:2], u[:, :, 2:3])
        nc.scalar.copy(u[:, :, WP - 2:WP - 1], u[:, :, WP - 3:WP - 2])
    yield

    # ---- 3-tap horizontal gradients (fp32, base-0) ----
    gx, gxf = gh, ghf        # gh dead after vertical+corner
    ay, ayf = wt("E")
    nc.vector.tensor_tensor(gxf[:, 0:LV], u1f[:, 3:3 + LV], u1f[:, 1:1 + LV],
                            AL.subtract)
    nc.gpsimd.tensor_tensor(ayf[:, 0:LV], u2f[:, 1:1 + LV], u2f[:, 3:3 + LV],
                            AL.add)
    nc.vector.scalar_tensor_tensor(ayf[:, 0:LV], u2f[:, 2:2 + LV], 2.0,
                                   ayf[:, 0:LV], AL.mult, AL.add)
    gy, gyf = ay, ayf
    yield

    # ---- squares (ACT fp32), s16 fp16 compare field, masks ----
    sx, sxf = u1, u1f        # u1 dead after gx
    sy, syf = u2, u2f        # u2 dead after gy
    nc.scalar.activation(sxf[:, 0:LV], gxf[:, 0:LV], AF.Square, 0.0, C0)
    nc.scalar.activation(syf[:, 0:LV], gyf[:, 0:LV], AF.Square, 0.0, C0)
    s16, s16f = wt16("S16")
    nc.vector.tensor_tensor(s16f[:, 2:2 + LV], sxf[:, 0:LV], syf[:, 0:LV],
                            AL.add)
    nc.gpsimd.memset(s16[:, :, 0:PAD], 0.0)
    nc.gpsimd.memset(s16[:, :, WP - PAD:WP], 0.0)
    # masks: notch = (T2*sx < sy), cv = (TH2*sx < sy)  [strict compares only]
    notch, notchf = wt16("NCH", U16)
    cvm, cvmf = wt16("CV", U16)
    nc.vector.scalar_tensor_tensor(notchf[:, 0:LV], sxf[:, 0:LV], T2,
                                   syf[:, 0:LV], AL.mult, AL.is_lt)
    nc.vector.scalar_tensor_tensor(cvmf[:, 0:LV], sxf[:, 0:LV], TH2,
                                   syf[:, 0:LV], AL.mult, AL.is_lt)
    # md1 = (gx*gy > 0): fp16 product then 4x-mode fp16 TS compare
    pxy, pxyf = wt16("PXY")
    nc.vector.tensor_tensor(pxyf[:, 0:LV], gxf[:, 0:LV], gyf[:, 0:LV],
                            AL.mult)
    md1, md1f = wt16("MD", U16)
    nc.vector.tensor_scalar(md1f[:, 0:LV], pxyf[:, 0:LV], 0.0, None, AL.is_gt)
    yield

    # ---- row-shifted planes U16[r]=s16[r+1], D16[r]=s16[r-1] (fp16 PE) ----
    U16t, U16f = wt16("U16")
    D16t, D16f = wt16("D16")
    for (ci, pl) in ((0, U16t), (1, D16t)):
        for h in range(2):
            ps = ppsumv.tile([P, 2, W], F32, tag="psv", name="psv")
            for k in range(2):
                t = 2 * h + k
                nc.tensor.matmul(ps[:, k, :], c_shm16[:, ci, :],
                                 s16[:, t, INT], start=True, stop=True)
            nc.scalar.copy(pl[:, 2 * h:2 * h + 2, INT], ps[:])
    for pl in (U16t, D16t):
        nc.gpsimd.memset(pl[:, :, 0:PAD], 0.0)
        nc.gpsimd.memset(pl[:, :, WP - PAD:WP], 0.0)
    # inter-block boundary rows (one consolidated DMA each) + edge zeros
    nc.gpsimd.dma_start(
        U16t[127:128, 0:NT - 1, INT],
        s16[0:1, 1:NT, INT])
    nc.gpsimd.dma_start(U16t[127:128, NT - 1, INT], zrow16[0:1, :])
    nc.gpsimd.dma_start(
        D16t[0:1, 1:NT, INT],
        s16[127:128, 0:NT - 1, INT])
    nc.gpsimd.memset(D16t[0:1, 0, INT], 0.0)
    yield

    # ---- pair maxes + select chain (fp16) ----
    selx, selxf = wt16("SELX")   # base m3, becomes diag/vert select
    m1t, m1tf = wt16("M1")
    mvt, mvtf = wt16("MV")
    sel, self_ = wt16("SEL")     # base mh, becomes final selection
    nc.vector.tensor_tensor(selxf[:, 0:LV], U16f[:, 1:1 + LV],
                            D16f[:, 3:3 + LV], AL.max)
    nc.vector.tensor_tensor(m1tf[:, 0:LV], U16f[:, 3:3 + LV],
                            D16f[:, 1:1 + LV], AL.max)
    nc.vector.tensor_tensor(mvtf[:, 0:LV], U16f[:, 2:2 + LV],
                            D16f[:, 2:2 + LV], AL.max)
    nc.vector.tensor_tensor(self_[:, 0:LV], s16f[:, 1:1 + LV],
                            s16f[:, 3:3 + LV], AL.max)
    nc.vector.copy_predicated(selxf[:, 0:LV], md1f[:, 0:LV], m1tf[:, 0:LV])
    nc.vector.copy_predicated(selxf[:, 0:LV], cvmf[:, 0:LV], mvtf[:, 0:LV])
    nc.vector.copy_predicated(self_[:, 0:LV], notchf[:, 0:LV], selxf[:, 0:LV])
    yield

    # ---- keep, magnitude, clip, store (fp16 out) ----
    keep, keepf = m1t, m1tf      # m1 consumed by first cp
    nc.vector.tensor_tensor(keepf[:, 0:LV], s16f[:, 2:2 + LV],
                            self_[:, 0:LV], AL.is_gt)
    mag, magf = mvt, mvtf        # mv consumed by second cp
    nc.scalar.activation(magf[:, 0:LV], s16f[:, 2:2 + LV], AF.Sqrt,
                         0.0, 1.0)
    nc.vector.tensor_scalar(magf[:, 0:LV], magf[:, 0:LV], 1.0, None, AL.min)
    out16, out16f = selx, selxf  # selx consumed by final cp
    nc.vector.tensor_tensor(out16f[:, 0:LV], magf[:, 0:LV], keepf[:, 0:LV],
                            AL.mult)
    nc.sync.dma_start(
        ydram[img].rearrange("(t p) w -> p t w", p=P),
        out16[:, :, 0:W])
    yield


def _build():
    import concourse.bacc as bacc
    import concourse.mybir as mybir
    from concourse import tile
    F32 = mybir.dt.float32
    F16 = mybir.dt.float16

    nc = bacc.Bacc("TRN2", target_bir_lowering=False, debug=False,
                   num_devices=NCORES)
    xdram = nc.declare_dram_parameter("xc", [NI, 3, H, W], F32, isOutput=False)
    c_vs_d = nc.declare_dram_parameter("vs", [P, NT, P], F32, isOutput=False)
    c_vd_d = nc.declare_dram_parameter("vd", [P, NT, P], F32, isOutput=False)
    c_vcor_d = nc.declare_dram_parameter("vcor", [36, 2, 18], F32,
                                         isOutput=False)
    c_shm_d = nc.declare_dram_parameter("shm16", [P, 2, P], F16,
                                        isOutput=False)
    ydram = nc.declare_dram_parameter("y", [NI, H, W], F16, isOutput=True)

    with tile.TileContext(nc) as tc:
        with tc.tile_pool(name="pconst", bufs=1) as pconst, \
             tc.tile_pool(name="pwork", bufs=2) as pwork, \
             tc.tile_pool(name="pw16", bufs=2) as pw16, \
             tc.tile_pool(name="psmall", bufs=2) as psmall, \
             tc.tile_pool(name="ppsumv", bufs=3, space="PSUM") as ppsumv, \
             tc.tile_pool(name="ppsumc", bufs=2, space="PSUM") as ppsumc:
            c_vs = pconst.tile([P, NT, P], F32, tag="cvs")
            nc.sync.dma_start(c_vs[:], c_vs_d[:])
            c_vd = pconst.tile([P, NT, P], F32, tag="cvd")
            nc.sync.dma_start(c_vd[:], c_vd_d[:])
            c_vcor = pconst.tile([36, 2, 18], F32, tag="cvcor")
            nc.sync.dma_start(c_vcor[:], c_vcor_d[:])
            c_shm16 = pconst.tile([P, 2, P], F16, tag="cshm")
            nc.sync.dma_start(c_shm16[:], c_shm_d[:])
            zrow16 = pconst.tile([1, W], F16, tag="zr16")
            nc.gpsimd.memset(zrow16[:], 0.0)

            pools = (pwork, pw16, psmall, (ppsumv, ppsumc))
            tens = (xdram, ydram, c_vs, c_vd, c_vcor, c_shm16, zrow16)
            import os
            nrep = int(os.environ.get("KREPEAT", "1"))
            for rep in range(nrep):
                gens = [_emit_image(nc, tc, pools, tens, img)
                        for img in range(NI)]
                done = [False] * NI
                while not all(done):
                    for i, gi in enumerate(gens):
                        if not done[i]:
                            try:
                                next(gi)
                            except StopIteration:
                                done[i] = True

    nc.compile()
    return nc


def _get_nc():
    if "nc" not in _CACHE:
        _CACHE["nc"] = _build()
        _CACHE["consts"] = _build_consts()
    return _CACHE["nc"], _CACHE["consts"]


def kernel(x):
    from concourse.bass_utils import run_bass_kernel_spmd
    x = np.ascontiguousarray(np.asarray(x), dtype=np.float32)
    assert x.shape == (16, 3, H, W), x.shape
    nc, consts = _get_nc()
    in_maps = []
    for c in range(NCORES):
        m = {"xc": x[NI * c:NI * (c + 1)]}
        m.update(consts)
        in_maps.append(m)
    res = run_bass_kernel_spmd(nc, in_maps, list(range(NCORES)))
    y = np.concatenate([res.results[c]["y"] for c in range(NCORES)], axis=0)
    y = y.astype(np.float32)[:, None]          # widen fp16 -> f32, add C dim
    return np.repeat(y, 3, axis=1)             # replicate 3 identical channels


if __name__ == "__main__":
    import golden
    rng = np.random.default_rng(0)
    x = rng.random((16, 3, H, W), dtype=np.float32)
    y = kernel(x)
    ref = golden.reference_np(x)
    d = y - ref
    print("L2 rel:", np.linalg.norm(d) / np.linalg.norm(ref))
    print("absmax:", np.abs(d).max(), " bigpix:", (np.abs(d) > 1e-3).sum())


# revision 13
# speedup vs baseline: 1.0802x; 1.0802x over previous
"""Trainium2 Bass kernel for CannyExtractor (NMS-suppressed canny magnitude).

Contract: kernel(x) takes FULL input x [16,3,512,512] f32, returns FULL output
[16,3,512,512] f32. Internally: batch sharded over 8 NeuronCores (2 images
per core), one SPMD Bass program, device emits the fp16 single-channel
suppressed magnitude; host widens to f32 and replicates the 3 identical
channels (reference output is channel-replicated).

Pipeline per image (fp32 until squares — fp16 compare field; the precision
split is forced: quantizing anything upstream of gx/gy to fp16 pushes L2
rel-err past the 2e-2 gate due to cancellation in the derivative taps):
  gray (DVE STT fp32) -> horizontal 5-tap gaussian (POOL adds + DVE STTs,
  fp32) -> vertical composite convs on PE (banded fp32 matmuls + corner
  matmul for inter-block halos, direct PSUM->SBUF flat relays on ACT) ->
  3-tap horizontal gradients (DVE/POOL fp32) -> squares on ACT (fp32) ->
  s16 fp16 compare field; NMS masks as strict-only compares (is_lt/is_gt;
  is_ge measured 3.2x slower than is_gt on DVE) -> row-shifted planes via
  fp16 PE matmuls -> pair maxes + copy_predicated select chain -> keep,
  sqrt (ACT, +eps bias), clip, apply -> fp16 out, one DMA per image.
"""
import sys
import numpy as np

sys.path.insert(0, "/opt/trn_rl_repo")

H = W = 512
NT = 4            # 128-row blocks per image
P = 128
PAD = 2
WP = W + 2 * PAD  # padded plane width (516)
L = NT * WP       # flat free length (2064)
LV = L - 4        # flat op length (2060)
INT = slice(PAD, PAD + W)
NI = 2            # images per core
NCORES = 8

GRAY = np.array([0.299, 0.587, 0.114], np.float32)
SQT2 = np.float32(np.sqrt(2.0) - 1.0)        # tan(22.5 deg)
T2 = float(np.float32(SQT2 * SQT2))          # tan^2(22.5)
TH2 = float(np.float32(1.0 / (SQT2 * SQT2)))  # tan^2(67.5)


def _gauss5():
    ax = np.arange(5, dtype=np.float32) - 2.0
    g1 = np.exp(-0.5 * ax * ax).astype(np.float32)
    return (g1 / g1.sum()).astype(np.float32)


def _vert_matrix(kind):
    g1 = _gauss5()
    I = np.eye(H, dtype=np.float64)
    X = np.pad(I, ((2, 2), (0, 0)), mode="reflect")
    B = np.zeros((H, H))
    for k in range(5):
        B += g1[k] * X[k:k + H]
    Y = np.pad(B, ((1, 1), (0, 0)), mode="edge")
    taps = [1.0, 2.0, 1.0] if kind == "smooth" else [-1.0, 0.0, 1.0]
    M = np.zeros((H, H))
    for k in range(3):
        if taps[k] != 0.0:
            M += taps[k] * Y[k:k + H]
    return M


def _build_consts():
    Ms = (_vert_matrix("smooth") * float(GRAY[2])).astype(np.float32)
    Md = (_vert_matrix("diff") * float(GRAY[2])).astype(np.float32)
    vs = np.zeros((P, NT, P), np.float32)
    vd = np.zeros((P, NT, P), np.float32)
    for t in range(NT):
        vs[:, t, :] = Ms[128 * t:128 * (t + 1), 128 * t:128 * (t + 1)].T
        vd[:, t, :] = Md[128 * t:128 * (t + 1), 128 * t:128 * (t + 1)].T
    vcor = np.zeros((36, 2, 18), np.float32)
    for b in range(3):
        in_rows = [128 * b + 122 + k for k in range(12)]
        out_rows = [128 * b + 125, 128 * b + 126, 128 * b + 127,
                    128 * (b + 1), 128 * (b + 1) + 1, 128 * (b + 1) + 2]
        for k, ir in enumerate(in_rows):
            for m, orr in enumerate(out_rows):
                vcor[12 * b + k, 0, 6 * b + m] = Ms[orr, ir]
                vcor[12 * b + k, 1, 6 * b + m] = Md[orr, ir]
    # fp16 row-shift matrices: up[k,m]=1 iff k=m+1 (U[m]=s[m+1]); dn: k=m-1
    shm = np.zeros((P, 2, P), np.float16)
    for m in range(P - 1):
        shm[m + 1, 0, m] = 1.0
    for m in range(1, P):
        shm[m - 1, 1, m] = 1.0
    return {"vs": vs, "vd": vd, "vcor": vcor, "shm16": shm}


_CACHE = {}


def _emit_image(nc, tc, pools, tens, img):
    """Generator: yields between stages so the caller interleaves 2 images."""
    import concourse.mybir as mybir
    AL = mybir.AluOpType
    AF = mybir.ActivationFunctionType
    F32 = mybir.dt.float32
    F16 = mybir.dt.float16
    U16 = mybir.dt.uint16

    pwork, pw16, psmall, (ppsumv, ppsumc) = pools
    (xdram, ydram, c_vs, c_vd, c_vcor, c_shm16, zrow16) = tens

    g1 = _gauss5()
    C0, C1, C2 = float(g1[2]), float(g1[1]), float(g1[0])
    R01 = float(np.float32(GRAY[0] / GRAY[1]))
    R12 = float(np.float32(GRAY[1] / GRAY[2]))

    def wt(tag):
        t = pwork.tile([P, NT, WP], F32, tag=tag, name=tag)
        return t, t[:].rearrange("p t w -> p (t w)")

    def wt16(tag, dt=F16):
        t = pw16.tile([P, NT, WP], dt, tag=tag, name=tag)
        return t, t[:].rearrange("p t w -> p (t w)")

    # ---- load input channels (1 DMA per channel, 3D AP) ----
    xa, xaf = wt("A")
    xb, xbf = wt("B")
    xc, xcf = wt("C")
    for h in range(2):
        for c, t in enumerate((xa, xb, xc)):
            nc.sync.dma_start(
                t[:, 2 * h:2 * h + 2, INT],
                xdram[img, c].rearrange("(t p) w -> p t w", p=P)[:, 2 * h:2 * h + 2, :])
    yield

    # ---- grayscale (fp32; scale 1/GRAY[2], folded into vertical mats) ----
    nc.vector.scalar_tensor_tensor(xbf[:, PAD:L], xaf[:, PAD:L],
                                   R01, xbf[:, PAD:L], AL.mult, AL.add)
    nc.vector.scalar_tensor_tensor(xcf[:, PAD:L], xbf[:, PAD:L],
                                   R12, xcf[:, PAD:L], AL.mult, AL.add)
    g, gf = xc, xcf          # gray plane, base-2
    nc.scalar.copy(g[:, :, 1:2], g[:, :, 3:4])
    nc.scalar.copy(g[:, :, 0:1], g[:, :, 4:5])
    nc.scalar.copy(g[:, :, WP - 2:WP - 1], g[:, :, WP - 4:WP - 3])
    nc.scalar.copy(g[:, :, WP - 1:WP], g[:, :, WP - 5:WP - 4])
    yield

    # ---- horizontal 5-tap gaussian (fp32), gh base-2 = blur/C0 ----
    a1, a1f = xa, xaf        # xa dead after gray STT1
    a2, a2f = xb, xbf        # xb dead after gray STT2
    nc.gpsimd.tensor_tensor(a1f[:, 0:LV], gf[:, 1:1 + LV], gf[:, 3:3 + LV],
                            AL.add)
    nc.gpsimd.tensor_tensor(a2f[:, 0:LV], gf[:, 0:LV], gf[:, 4:4 + LV],
                            AL.add)
    nc.vector.scalar_tensor_tensor(a1f[:, 0:LV], a2f[:, 0:LV], C2 / C1,
                                   a1f[:, 0:LV], AL.mult, AL.add)
    gh, ghf = wt("D")
    nc.vector.scalar_tensor_tensor(ghf[:, 2:2 + LV], a1f[:, 0:LV], C1 / C0,
                                   gf[:, 2:2 + LV], AL.mult, AL.add)
    yield

    # ---- vertical composite convs on PE (fp32) ----
    # corner-halo chain first so it overlaps the main banded matmuls
    u1, u1f = xa, xaf        # a1 dead after gh
    u2, u2f = xb, xbf        # a2 dead after gh
    cs = psmall.tile([36, W], F32, tag="cs", name="cs")
    for b in range(3):
        nc.sync.dma_start(cs[12 * b:12 * b + 6, :], gh[122:128, b, INT])
        nc.sync.dma_start(cs[12 * b + 6:12 * b + 12, :], gh[0:6, b + 1, INT])
    cos = []
    for ci in (0, 1):
        cps = ppsumc.tile([18, W], F32, tag="cps", name="cps")
        nc.tensor.matmul(cps[:], c_vcor[:, ci, :], cs[:], start=True, stop=True)
        co = psmall.tile([18, W], F32, tag="co", name="co")
        nc.scalar.copy(co[:], cps[:])
        cos.append(co)
    for (cm, u) in ((c_vs, u1), (c_vd, u2)):
        for h in range(2):
            ps = ppsumv.tile([P, 2, W], F32, tag="psv", name="psv")
            for k in range(2):
                t = 2 * h + k
                nc.tensor.matmul(ps[:, k, :], cm[:, t, :], gh[:, t, INT],
                                 start=True, stop=True)
            nc.scalar.copy(u[:, 2 * h:2 * h + 2, INT], ps[:])
    for ci, u in ((0, u1), (1, u2)):
        co = cos[ci]
        for b in range(3):
            nc.sync.dma_start(u[125:128, b, INT], co[6 * b:6 * b + 3, :])
            nc.sync.dma_start(u[0:3, b + 1, INT], co[6 * b + 3:6 * b + 6, :])
    for u in (u1, u2):
        nc.scalar.copy(u[:, :, 1:2], u[:, :, 2:3])
        nc.scalar.copy(u[:, :, WP - 2:WP - 1], u[:, :, WP - 3:WP - 2])
    yield

    # ---- 3-tap horizontal gradients (fp32, base-0) ----
    gx, gxf = gh, ghf        # gh dead after vertical+corner
    ay, ayf = wt("E")
    nc.vector.tensor_tensor(gxf[:, 0:LV], u1f[:, 3:3 + LV], u1f[:, 1:1 + LV],
                            AL.subtract)
    nc.gpsimd.tensor_tensor(ayf[:, 0:LV], u2f[:, 1:1 + LV], u2f[:, 3:3 + LV],
                            AL.add)
    nc.vector.scalar_tensor_tensor(ayf[:, 0:LV], u2f[:, 2:2 + LV], 2.0,
                                   ayf[:, 0:LV], AL.mult, AL.add)
    gy, gyf = ay, ayf
    yield

    # ---- squares (ACT fp32), s16 fp16 compare field, masks ----
    sx, sxf = u1, u1f        # u1 dead after gx
    sy, syf = u2, u2f        # u2 dead after gy
    nc.scalar.activation(sxf[:, 0:LV], gxf[:, 0:LV], AF.Square, 0.0, C0)
    nc.scalar.activation(syf[:, 0:LV], gyf[:, 0:LV], AF.Square, 0.0, C0)
    s16, s16f = wt16("S16")
    nc.vector.tensor_tensor(s16f[:, 2:2 + LV], sxf[:, 0:LV], syf[:, 0:LV],
                            AL.add)
    nc.gpsimd.memset(s16[:, :, 0:PAD], 0.0)
    nc.gpsimd.memset(s16[:, :, WP - PAD:WP], 0.0)
    # masks: notch = (T2*sx < sy), cv = (TH2*sx < sy)  [strict compares only]
    notch, notchf = wt16("NCH", U16)
    cvm, cvmf = wt16("CV", U16)
    nc.vector.scalar_tensor_tensor(notchf[:, 0:LV], sxf[:, 0:LV], T2,
                                   syf[:, 0:LV], AL.mult, AL.is_lt)
    nc.vector.scalar_tensor_tensor(cvmf[:, 0:LV], sxf[:, 0:LV], TH2,
                                   syf[:, 0:LV], AL.mult, AL.is_lt)
    # md1 = (gx*gy > 0): fp16 product then 4x-mode fp16 TS compare
    pxy, pxyf = wt16("PXY")
    nc.vector.tensor_tensor(pxyf[:, 0:LV], gxf[:, 0:LV], gyf[:, 0:LV],
                            AL.mult)
    md1, md1f = wt16("MD", U16)
    nc.vector.tensor_scalar(md1f[:, 0:LV], pxyf[:, 0:LV], 0.0, None, AL.is_gt)
    yield

    # ---- row-shifted planes U16[r]=s16[r+1], D16[r]=s16[r-1] (fp16 PE) ----
    U16t, U16f = wt16("U16")
    D16t, D16f = wt16("D16")
    for (ci, pl) in ((0, U16t), (1, D16t)):
        for h in range(2):
            ps = ppsumv.tile([P, 2, W], F32, tag="psv", name="psv")
            for k in range(2):
                t = 2 * h + k
                nc.tensor.matmul(ps[:, k, :], c_shm16[:, ci, :],
                                 s16[:, t, INT], start=True, stop=True)
            nc.scalar.copy(pl[:, 2 * h:2 * h + 2, INT], ps[:])
    for pl in (U16t, D16t):
        nc.gpsimd.memset(pl[:, :, 0:PAD], 0.0)
        nc.gpsimd.memset(pl[:, :, WP - PAD:WP], 0.0)
    # inter-block boundary rows (one consolidated DMA each) + edge zeros
    nc.sync.dma_start(
        U16t[127:128, 0:NT - 1, INT],
        s16[0:1, 1:NT, INT])
    nc.sync.dma_start(U16t[127:128, NT - 1, INT], zrow16[0:1, :])
    nc.sync.dma_start(
        D16t[0:1, 1:NT, INT],
        s16[127:128, 0:NT - 1, INT])
    nc.gpsimd.memset(D16t[0:1, 0, INT], 0.0)
    yield

    # ---- pair maxes + select chain (fp16) ----
    selx, selxf = wt16("SELX")   # base m3, becomes diag/vert select
    m1t, m1tf = wt16("M1")
    mvt, mvtf = wt16("MV")
    sel, self_ = wt16("SEL")     # base mh, becomes final selection
    nc.vector.tensor_tensor(selxf[:, 0:LV], U16f[:, 1:1 + LV],
                            D16f[:, 3:3 + LV], AL.max)
    nc.vector.tensor_tensor(m1tf[:, 0:LV], U16f[:, 3:3 + LV],
                            D16f[:, 1:1 + LV], AL.max)
    nc.vector.tensor_tensor(mvtf[:, 0:LV], U16f[:, 2:2 + LV],
                            D16f[:, 2:2 + LV], AL.max)
    nc.vector.tensor_tensor(self_[:, 0:LV], s16f[:, 1:1 + LV],
                            s16f[:, 3:3 + LV], AL.max)
    nc.vector.copy_predicated(selxf[:, 0:LV], md1f[:, 0:LV], m1tf[:, 0:LV])
    nc.vector.copy_predicated(selxf[:, 0:LV], cvmf[:, 0:LV], mvtf[:, 0:LV])
    nc.vector.copy_predicated(self_[:, 0:LV], notchf[:, 0:LV], selxf[:, 0:LV])
    yield

    # ---- keep, magnitude, clip, store (fp16 out) ----
    keep, keepf = m1t, m1tf      # m1 consumed by first cp
    nc.vector.tensor_tensor(keepf[:, 0:LV], s16f[:, 2:2 + LV],
                            self_[:, 0:LV], AL.is_gt)
    mag, magf = mvt, mvtf        # mv consumed by second cp
    nc.scalar.activation(magf[:, 0:LV], s16f[:, 2:2 + LV], AF.Sqrt,
                         0.0, 1.0)
    nc.vector.tensor_scalar(magf[:, 0:LV], magf[:, 0:LV], 1.0, None, AL.min)
    out16, out16f = selx, selxf  # selx consumed by final cp
    nc.vector.tensor_tensor(out16f[:, 0:LV], magf[:, 0:LV], keepf[:, 0:LV],
                            AL.mult)
    nc.sync.dma_start(
        ydram[img].rearrange("(t p) w -> p t w", p=P),
        out16[:, :, 0:W])
    yield


def _build():
    import concourse.bacc as bacc
    import concourse.mybir as mybir
    from concourse import tile
    F32 = mybir.dt.float32
    F16 = mybir.dt.float16

    nc = bacc.Bacc("TRN2", target_bir_lowering=False, debug=False,
                   num_devices=NCORES)
    xdram = nc.declare_dram_parameter("xc", [NI, 3, H, W], F32, isOutput=False)
    c_vs_d = nc.declare_dram_parameter("vs", [P, NT, P], F32, isOutput=False)
    c_vd_d = nc.declare_dram_parameter("vd", [P, NT, P], F32, isOutput=False)
    c_vcor_d = nc.declare_dram_parameter("vcor", [36, 2, 18], F32,
                                         isOutput=False)
    c_shm_d = nc.declare_dram_parameter("shm16", [P, 2, P], F16,
                                        isOutput=False)
    ydram = nc.declare_dram_parameter("y", [NI, H, W], F16, isOutput=True)

    with tile.TileContext(nc) as tc:
        with tc.tile_pool(name="pconst", bufs=1) as pconst, \
             tc.tile_pool(name="pwork", bufs=2) as pwork, \
             tc.tile_pool(name="pw16", bufs=2) as pw16, \
             tc.tile_pool(name="psmall", bufs=2) as psmall, \
             tc.tile_pool(name="ppsumv", bufs=3, space="PSUM") as ppsumv, \
             tc.tile_pool(name="ppsumc", bufs=2, space="PSUM") as ppsumc:
            c_vs = pconst.tile([P, NT, P], F32, tag="cvs")
            nc.sync.dma_start(c_vs[:], c_vs_d[:])
            c_vd = pconst.tile([P, NT, P], F32, tag="cvd")
            nc.sync.dma_start(c_vd[:], c_vd_d[:])
            c_vcor = pconst.tile([36, 2, 18], F32, tag="cvcor")
            nc.sync.dma_start(c_vcor[:], c_vcor_d[:])
            c_shm16 = pconst.tile([P, 2, P], F16, tag="cshm")
            nc.sync.dma_start(c_shm16[:], c_shm_d[:])
            zrow16 = pconst.tile([1, W], F16, tag="zr16")
            nc.gpsimd.memset(zrow16[:], 0.0)

            pools = (pwork, pw16, psmall, (ppsumv, ppsumc))
            tens = (xdram, ydram, c_vs, c_vd, c_vcor, c_shm16, zrow16)
            import os
            nrep = int(os.environ.get("KREPEAT", "1"))
            for rep in range(nrep):
                gens = [_emit_image(nc, tc, pools, tens, img)
                        for img in range(NI)]
                done = [False] * NI
                while not all(done):
                    for i, gi in enumerate(gens):
                        if not done[i]:
                            try:
                                next(gi)
                            except StopIteration:
                                done[i] = True

    nc.compile()
    return nc


def _get_nc():
    if "nc" not in _CACHE:
        _CACHE["nc"] = _build()
        _CACHE["consts"] = _build_consts()
    return _CACHE["nc"], _CACHE["consts"]


def kernel(x):
    from concourse.bass_utils import run_bass_kernel_spmd
    x = np.ascontiguousarray(np.asarray(x), dtype=np.float32)
    assert x.shape == (16, 3, H, W), x.shape
    nc, consts = _get_nc()
    in_maps = []
    for c in range(NCORES):
        m = {"xc": x[NI * c:NI * (c + 1)]}
        m.update(consts)
        in_maps.append(m)
    res = run_bass_kernel_spmd(nc, in_maps, list(range(NCORES)))
    y = np.concatenate([res.results[c]["y"] for c in range(NCORES)], axis=0)
    y = y.astype(np.float32)[:, None]          # widen fp16 -> f32, add C dim
    return np.repeat(y, 3, axis=1)             # replicate 3 identical channels


if __name__ == "__main__":
    import golden
    rng = np.random.default_rng(0)
    x = rng.random((16, 3, H, W), dtype=np.float32)
    y = kernel(x)
    ref = golden.reference_np(x)
    d = y - ref
    print("L2 rel:", np.linalg.norm(d) / np.linalg.norm(ref))
    print("absmax:", np.abs(d).max(), " bigpix:", (np.abs(d) > 1e-3).sum())


# revision 14
# speedup vs baseline: 1.2804x; 1.1853x over previous
"""Trainium2 Bass kernel for CannyExtractor (NMS-suppressed canny magnitude).

Contract: kernel(x) takes FULL input x [16,3,512,512] f32, returns FULL output
[16,3,512,512] f32. Internally: batch sharded over 8 NeuronCores (2 images
per core), one SPMD Bass program, device emits the fp16 single-channel
suppressed magnitude; host widens to f32 and replicates the 3 identical
channels (reference output is channel-replicated).

Pipeline per image (fp32 until squares — fp16 compare field; the precision
split is forced: quantizing anything upstream of gx/gy to fp16 pushes L2
rel-err past the 2e-2 gate due to cancellation in the derivative taps):
  gray (DVE STT fp32) -> horizontal 5-tap gaussian (POOL adds + DVE STTs,
  fp32) -> vertical composite convs on PE (banded fp32 matmuls + corner
  matmul for inter-block halos, direct PSUM->SBUF flat relays on ACT) ->
  3-tap horizontal gradients (DVE/POOL fp32) -> squares on ACT (fp32) ->
  s16 fp16 compare field; NMS masks as strict-only compares (is_lt/is_gt;
  is_ge measured 3.2x slower than is_gt on DVE) -> row-shifted planes via
  fp16 PE matmuls -> pair maxes + copy_predicated select chain -> keep,
  sqrt (ACT, +eps bias), clip, apply -> fp16 out, one DMA per image.
"""
import sys
import numpy as np

sys.path.insert(0, "/opt/trn_rl_repo")

H = W = 512
NT = 4            # 128-row blocks per image
P = 128
PAD = 2
WP = W + 2 * PAD  # padded plane width (516)
L = NT * WP       # flat free length (2064)
LV = L - 4        # flat op length (2060)
INT = slice(PAD, PAD + W)
NI = 2            # images per core
NCORES = 8

GRAY = np.array([0.299, 0.587, 0.114], np.float32)
SQT2 = np.float32(np.sqrt(2.0) - 1.0)        # tan(22.5 deg)
T2 = float(np.float32(SQT2 * SQT2))          # tan^2(22.5)
TH2 = float(np.float32(1.0 / (SQT2 * SQT2)))  # tan^2(67.5)


def _gauss5():
    ax = np.arange(5, dtype=np.float32) - 2.0
    g1 = np.exp(-0.5 * ax * ax).astype(np.float32)
    return (g1 / g1.sum()).astype(np.float32)


def _vert_matrix(kind):
    g1 = _gauss5()
    I = np.eye(H, dtype=np.float64)
    X = np.pad(I, ((2, 2), (0, 0)), mode="reflect")
    B = np.zeros((H, H))
    for k in range(5):
        B += g1[k] * X[k:k + H]
    Y = np.pad(B, ((1, 1), (0, 0)), mode="edge")
    taps = [1.0, 2.0, 1.0] if kind == "smooth" else [-1.0, 0.0, 1.0]
    M = np.zeros((H, H))
    for k in range(3):
        if taps[k] != 0.0:
            M += taps[k] * Y[k:k + H]
    return M


def _build_consts():
    Ms = (_vert_matrix("smooth") * float(GRAY[2])).astype(np.float32)
    Md = (_vert_matrix("diff") * float(GRAY[2])).astype(np.float32)
    vs = np.zeros((P, NT, P), np.float32)
    vd = np.zeros((P, NT, P), np.float32)
    for t in range(NT):
        vs[:, t, :] = Ms[128 * t:128 * (t + 1), 128 * t:128 * (t + 1)].T
        vd[:, t, :] = Md[128 * t:128 * (t + 1), 128 * t:128 * (t + 1)].T
    vcor = np.zeros((36, 2, 18), np.float32)
    for b in range(3):
        in_rows = [128 * b + 122 + k for k in range(12)]
        out_rows = [128 * b + 125, 128 * b + 126, 128 * b + 127,
                    128 * (b + 1), 128 * (b + 1) + 1, 128 * (b + 1) + 2]
        for k, ir in enumerate(in_rows):
            for m, orr in enumerate(out_rows):
                vcor[12 * b + k, 0, 6 * b + m] = Ms[orr, ir]
                vcor[12 * b + k, 1, 6 * b + m] = Md[orr, ir]
    # fp16 row-shift matrices: up[k,m]=1 iff k=m+1 (U[m]=s[m+1]); dn: k=m-1
    shm = np.zeros((P, 2, P), np.float16)
    for m in range(P - 1):
        shm[m + 1, 0, m] = 1.0
    for m in range(1, P):
        shm[m - 1, 1, m] = 1.0
    return {"vs": vs, "vd": vd, "vcor": vcor, "shm16": shm}


_CACHE = {}


def _emit_image(nc, tc, pools, tens, img):
    """Generator: yields between stages so the caller interleaves 2 images."""
    import concourse.mybir as mybir
    AL = mybir.AluOpType
    AF = mybir.ActivationFunctionType
    F32 = mybir.dt.float32
    F16 = mybir.dt.float16
    U16 = mybir.dt.uint16

    pwork, pw16, psmall, (ppsumv, ppsumc) = pools
    (xdram, ydram, c_vs, c_vd, c_vcor, c_shm16, zrow16) = tens

    g1 = _gauss5()
    C0, C1, C2 = float(g1[2]), float(g1[1]), float(g1[0])
    R01 = float(np.float32(GRAY[0] / GRAY[1]))
    R12 = float(np.float32(GRAY[1] / GRAY[2]))

    def wt(tag):
        t = pwork.tile([P, NT, WP], F32, tag=tag, name=tag)
        return t, t[:].rearrange("p t w -> p (t w)")

    def wt16(tag, dt=F16):
        t = pw16.tile([P, NT, WP], dt, tag=tag, name=tag)
        return t, t[:].rearrange("p t w -> p (t w)")

    # ---- load input channels (1 DMA per channel, 3D AP) ----
    xa, xaf = wt("A")
    xb, xbf = wt("B")
    xc, xcf = wt("C")
    for c, t in enumerate((xa, xb, xc)):
        nc.sync.dma_start(
            t[:, :, INT],
            xdram[img, c].rearrange("(t p) w -> p t w", p=P))
    yield

    # ---- grayscale (fp32; scale 1/GRAY[2], folded into vertical mats) ----
    nc.vector.scalar_tensor_tensor(xbf[:, PAD:L], xaf[:, PAD:L],
                                   R01, xbf[:, PAD:L], AL.mult, AL.add)
    nc.vector.scalar_tensor_tensor(xcf[:, PAD:L], xbf[:, PAD:L],
                                   R12, xcf[:, PAD:L], AL.mult, AL.add)
    g, gf = xc, xcf          # gray plane, base-2
    nc.scalar.copy(g[:, :, 1:2], g[:, :, 3:4])
    nc.scalar.copy(g[:, :, 0:1], g[:, :, 4:5])
    nc.scalar.copy(g[:, :, WP - 2:WP - 1], g[:, :, WP - 4:WP - 3])
    nc.scalar.copy(g[:, :, WP - 1:WP], g[:, :, WP - 5:WP - 4])
    yield

    # ---- horizontal 5-tap gaussian (fp32), gh base-2 = blur/C0 ----
    a1, a1f = xa, xaf        # xa dead after gray STT1
    a2, a2f = xb, xbf        # xb dead after gray STT2
    nc.gpsimd.tensor_tensor(a1f[:, 0:LV], gf[:, 1:1 + LV], gf[:, 3:3 + LV],
                            AL.add)
    nc.gpsimd.tensor_tensor(a2f[:, 0:LV], gf[:, 0:LV], gf[:, 4:4 + LV],
                            AL.add)
    nc.vector.scalar_tensor_tensor(a1f[:, 0:LV], a2f[:, 0:LV], C2 / C1,
                                   a1f[:, 0:LV], AL.mult, AL.add)
    gh, ghf = wt("D")
    nc.vector.scalar_tensor_tensor(ghf[:, 2:2 + LV], a1f[:, 0:LV], C1 / C0,
                                   gf[:, 2:2 + LV], AL.mult, AL.add)
    yield

    # ---- vertical composite convs on PE (fp32) ----
    # corner-halo chain first so it overlaps the main banded matmuls
    u1, u1f = xa, xaf        # a1 dead after gh
    u2, u2f = xb, xbf        # a2 dead after gh
    cs = psmall.tile([36, W], F32, tag="cs", name="cs")
    for b in range(3):
        nc.sync.dma_start(cs[12 * b:12 * b + 6, :], gh[122:128, b, INT])
        nc.sync.dma_start(cs[12 * b + 6:12 * b + 12, :], gh[0:6, b + 1, INT])
    cos = []
    for ci in (0, 1):
        cps = ppsumc.tile([18, W], F32, tag="cps", name="cps")
        nc.tensor.matmul(cps[:], c_vcor[:, ci, :], cs[:], start=True, stop=True)
        co = psmall.tile([18, W], F32, tag="co", name="co")
        nc.scalar.copy(co[:], cps[:])
        cos.append(co)
    for (cm, u) in ((c_vs, u1), (c_vd, u2)):
        for h in range(2):
            ps = ppsumv.tile([P, 2, W], F32, tag="psv", name="psv")
            for k in range(2):
                t = 2 * h + k
                nc.tensor.matmul(ps[:, k, :], cm[:, t, :], gh[:, t, INT],
                                 start=True, stop=True)
            nc.scalar.copy(u[:, 2 * h:2 * h + 2, INT], ps[:])
    for ci, u in ((0, u1), (1, u2)):
        co = cos[ci]
        for b in range(3):
            nc.sync.dma_start(u[125:128, b, INT], co[6 * b:6 * b + 3, :])
            nc.sync.dma_start(u[0:3, b + 1, INT], co[6 * b + 3:6 * b + 6, :])
    for u in (u1, u2):
        nc.scalar.copy(u[:, :, 1:2], u[:, :, 2:3])
        nc.scalar.copy(u[:, :, WP - 2:WP - 1], u[:, :, WP - 3:WP - 2])
    yield

    # ---- 3-tap horizontal gradients (fp32, base-0) ----
    gx, gxf = gh, ghf        # gh dead after vertical+corner
    ay, ayf = wt("E")
    nc.vector.tensor_tensor(gxf[:, 0:LV], u1f[:, 3:3 + LV], u1f[:, 1:1 + LV],
                            AL.subtract)
    nc.gpsimd.tensor_tensor(ayf[:, 0:LV], u2f[:, 1:1 + LV], u2f[:, 3:3 + LV],
                            AL.add)
    nc.vector.scalar_tensor_tensor(ayf[:, 0:LV], u2f[:, 2:2 + LV], 2.0,
                                   ayf[:, 0:LV], AL.mult, AL.add)
    gy, gyf = ay, ayf
    yield

    # ---- squares (ACT fp32), s16 fp16 compare field, masks ----
    sx, sxf = u1, u1f        # u1 dead after gx
    sy, syf = u2, u2f        # u2 dead after gy
    nc.scalar.activation(sxf[:, 0:LV], gxf[:, 0:LV], AF.Square, 0.0, C0)
    nc.scalar.activation(syf[:, 0:LV], gyf[:, 0:LV], AF.Square, 0.0, C0)
    s16, s16f = wt16("S16")
    nc.vector.tensor_tensor(s16f[:, 2:2 + LV], sxf[:, 0:LV], syf[:, 0:LV],
                            AL.add)
    nc.gpsimd.memset(s16[:, :, 0:PAD], 0.0)
    nc.gpsimd.memset(s16[:, :, WP - PAD:WP], 0.0)
    # masks: notch = (T2*sx < sy), cv = (TH2*sx < sy)  [strict compares only]
    notch, notchf = wt16("NCH", U16)
    cvm, cvmf = wt16("CV", U16)
    nc.vector.scalar_tensor_tensor(notchf[:, 0:LV], sxf[:, 0:LV], T2,
                                   syf[:, 0:LV], AL.mult, AL.is_lt)
    nc.vector.scalar_tensor_tensor(cvmf[:, 0:LV], sxf[:, 0:LV], TH2,
                                   syf[:, 0:LV], AL.mult, AL.is_lt)
    # md1 = (gx*gy > 0): fp16 product then 4x-mode fp16 TS compare
    pxy, pxyf = wt16("PXY")
    nc.vector.tensor_tensor(pxyf[:, 0:LV], gxf[:, 0:LV], gyf[:, 0:LV],
                            AL.mult)
    md1, md1f = wt16("MD", U16)
    nc.vector.tensor_scalar(md1f[:, 0:LV], pxyf[:, 0:LV], 0.0, None, AL.is_gt)
    yield

    # ---- row-shifted planes U16[r]=s16[r+1], D16[r]=s16[r-1] (fp16 PE) ----
    U16t, U16f = wt16("U16")
    D16t, D16f = wt16("D16")
    for (ci, pl) in ((0, U16t), (1, D16t)):
        for h in range(2):
            ps = ppsumv.tile([P, 2, W], F32, tag="psv", name="psv")
            for k in range(2):
                t = 2 * h + k
                nc.tensor.matmul(ps[:, k, :], c_shm16[:, ci, :],
                                 s16[:, t, INT], start=True, stop=True)
            nc.scalar.copy(pl[:, 2 * h:2 * h + 2, INT], ps[:])
    for pl in (U16t, D16t):
        nc.gpsimd.memset(pl[:, :, 0:PAD], 0.0)
        nc.gpsimd.memset(pl[:, :, WP - PAD:WP], 0.0)
    # inter-block boundary rows (one consolidated DMA each) + edge zeros
    nc.sync.dma_start(
        U16t[127:128, 0:NT - 1, INT],
        s16[0:1, 1:NT, INT])
    nc.sync.dma_start(U16t[127:128, NT - 1, INT], zrow16[0:1, :])
    nc.sync.dma_start(
        D16t[0:1, 1:NT, INT],
        s16[127:128, 0:NT - 1, INT])
    nc.gpsimd.memset(D16t[0:1, 0, INT], 0.0)
    yield

    # ---- pair maxes + select chain (fp16) ----
    selx, selxf = wt16("SELX")   # base m3, becomes diag/vert select
    m1t, m1tf = wt16("M1")
    mvt, mvtf = wt16("MV")
    sel, self_ = wt16("SEL")     # base mh, becomes final selection
    nc.vector.tensor_tensor(selxf[:, 0:LV], U16f[:, 1:1 + LV],
                            D16f[:, 3:3 + LV], AL.max)
    nc.vector.tensor_tensor(m1tf[:, 0:LV], U16f[:, 3:3 + LV],
                            D16f[:, 1:1 + LV], AL.max)
    nc.vector.tensor_tensor(mvtf[:, 0:LV], U16f[:, 2:2 + LV],
                            D16f[:, 2:2 + LV], AL.max)
    nc.vector.tensor_tensor(self_[:, 0:LV], s16f[:, 1:1 + LV],
                            s16f[:, 3:3 + LV], AL.max)
    nc.vector.copy_predicated(selxf[:, 0:LV], md1f[:, 0:LV], m1tf[:, 0:LV])
    nc.vector.copy_predicated(selxf[:, 0:LV], cvmf[:, 0:LV], mvtf[:, 0:LV])
    nc.vector.copy_predicated(self_[:, 0:LV], notchf[:, 0:LV], selxf[:, 0:LV])
    yield

    # ---- keep, magnitude, clip, store (fp16 out) ----
    keep, keepf = m1t, m1tf      # m1 consumed by first cp
    nc.vector.tensor_tensor(keepf[:, 0:LV], s16f[:, 2:2 + LV],
                            self_[:, 0:LV], AL.is_gt)
    mag, magf = mvt, mvtf        # mv consumed by second cp
    nc.scalar.activation(magf[:, 0:LV], s16f[:, 2:2 + LV], AF.Sqrt,
                         0.0, 1.0)
    nc.vector.tensor_scalar(magf[:, 0:LV], magf[:, 0:LV], 1.0, None, AL.min)
    out16, out16f = selx, selxf  # selx consumed by final cp
    nc.vector.tensor_tensor(out16f[:, 0:LV], magf[:, 0:LV], keepf[:, 0:LV],
                            AL.mult)
    nc.sync.dma_start(
        ydram[img].rearrange("(t p) w -> p t w", p=P),
        out16[:, :, 0:W])
    yield


def _build():
    import concourse.bacc as bacc
    import concourse.mybir as mybir
    from concourse import tile
    F32 = mybir.dt.float32
    F16 = mybir.dt.float16

    nc = bacc.Bacc("TRN2", target_bir_lowering=False, debug=False,
                   num_devices=NCORES)
    xdram = nc.declare_dram_parameter("xc", [NI, 3, H, W], F32, isOutput=False)
    c_vs_d = nc.declare_dram_parameter("vs", [P, NT, P], F32, isOutput=False)
    c_vd_d = nc.declare_dram_parameter("vd", [P, NT, P], F32, isOutput=False)
    c_vcor_d = nc.declare_dram_parameter("vcor", [36, 2, 18], F32,
                                         isOutput=False)
    c_shm_d = nc.declare_dram_parameter("shm16", [P, 2, P], F16,
                                        isOutput=False)
    ydram = nc.declare_dram_parameter("y", [NI, H, W], F16, isOutput=True)

    with tile.TileContext(nc) as tc:
        with tc.tile_pool(name="pconst", bufs=1) as pconst, \
             tc.tile_pool(name="pwork", bufs=2) as pwork, \
             tc.tile_pool(name="pw16", bufs=2) as pw16, \
             tc.tile_pool(name="psmall", bufs=2) as psmall, \
             tc.tile_pool(name="ppsumv", bufs=3, space="PSUM") as ppsumv, \
             tc.tile_pool(name="ppsumc", bufs=2, space="PSUM") as ppsumc:
            c_vs = pconst.tile([P, NT, P], F32, tag="cvs")
            nc.sync.dma_start(c_vs[:], c_vs_d[:])
            c_vd = pconst.tile([P, NT, P], F32, tag="cvd")
            nc.sync.dma_start(c_vd[:], c_vd_d[:])
            c_vcor = pconst.tile([36, 2, 18], F32, tag="cvcor")
            nc.sync.dma_start(c_vcor[:], c_vcor_d[:])
            c_shm16 = pconst.tile([P, 2, P], F16, tag="cshm")
            nc.sync.dma_start(c_shm16[:], c_shm_d[:])
            zrow16 = pconst.tile([1, W], F16, tag="zr16")
            nc.gpsimd.memset(zrow16[:], 0.0)

            pools = (pwork, pw16, psmall, (ppsumv, ppsumc))
            tens = (xdram, ydram, c_vs, c_vd, c_vcor, c_shm16, zrow16)
            import os
            nrep = int(os.environ.get("KREPEAT", "1"))
            for rep in range(nrep):
                gens = [_emit_image(nc, tc, pools, tens, img)
                        for img in range(NI)]
                done = [False] * NI
                while not all(done):
                    for i, gi in enumerate(gens):
                        if not done[i]:
                            try:
                                next(gi)
                            except StopIteration:
                                done[i] = True

    nc.compile()
    return nc


def _get_nc():
    if "nc" not in _CACHE:
        _CACHE["nc"] = _build()
        _CACHE["consts"] = _build_consts()
    return _CACHE["nc"], _CACHE["consts"]


def kernel(x):
    from concourse.bass_utils import run_bass_kernel_spmd
    x = np.ascontiguousarray(np.asarray(x), dtype=np.float32)
    assert x.shape == (16, 3, H, W), x.shape
    nc, consts = _get_nc()
    in_maps = []
    for c in range(NCORES):
        m = {"xc": x[NI * c:NI * (c + 1)]}
        m.update(consts)
        in_maps.append(m)
    res = run_bass_kernel_spmd(nc, in_maps, list(range(NCORES)))
    y = np.concatenate([res.results[c]["y"] for c in range(NCORES)], axis=0)
    y = y.astype(np.float32)[:, None]          # widen fp16 -> f32, add C dim
    return np.repeat(y, 3, axis=1)             # replicate 3 identical channels


if __name__ == "__main__":
    import golden
    rng = np.random.default_rng(0)
    x = rng.random((16, 3, H, W), dtype=np.float32)
    y = kernel(x)
    ref = golden.reference_np(x)
    d = y - ref
    print("L2 rel:", np.linalg.norm(d) / np.linalg.norm(ref))
    print("absmax:", np.abs(d).max(), " bigpix:", (np.abs(d) > 1e-3).sum())


# revision 15
# speedup vs baseline: 1.2917x; 1.0088x over previous
"""Trainium2 Bass kernel for CannyExtractor (NMS-suppressed canny magnitude).

Contract: kernel(x) takes FULL input x [16,3,512,512] f32, returns FULL output
[16,3,512,512] f32. Internally: batch sharded over 8 NeuronCores (2 images
per core), one SPMD Bass program, device emits the fp16 single-channel
suppressed magnitude; host widens to f32 and replicates the 3 identical
channels (reference output is channel-replicated).

Pipeline per image (fp32 until squares — fp16 compare field; the precision
split is forced: quantizing anything upstream of gx/gy to fp16 pushes L2
rel-err past the 2e-2 gate due to cancellation in the derivative taps):
  gray (DVE STT fp32) -> horizontal 5-tap gaussian (POOL adds + DVE STTs,
  fp32) -> vertical composite convs on PE (banded fp32 matmuls + corner
  matmul for inter-block halos, direct PSUM->SBUF flat relays on ACT) ->
  3-tap horizontal gradients (DVE/POOL fp32) -> squares on ACT (fp32) ->
  s16 fp16 compare field; NMS masks as strict-only compares (is_lt/is_gt;
  is_ge measured 3.2x slower than is_gt on DVE) -> row-shifted planes via
  fp16 PE matmuls -> pair maxes + copy_predicated select chain -> keep,
  sqrt (ACT, +eps bias), clip, apply -> fp16 out, one DMA per image.
"""
import sys
import numpy as np

sys.path.insert(0, "/opt/trn_rl_repo")

H = W = 512
NT = 4            # 128-row blocks per image
P = 128
PAD = 2
WP = W + 2 * PAD  # padded plane width (516)
L = NT * WP       # flat free length (2064)
LV = L - 4        # flat op length (2060)
INT = slice(PAD, PAD + W)
NI = 2            # images per core
NCORES = 8

GRAY = np.array([0.299, 0.587, 0.114], np.float32)
SQT2 = np.float32(np.sqrt(2.0) - 1.0)        # tan(22.5 deg)
T2 = float(np.float32(SQT2 * SQT2))          # tan^2(22.5)
TH2 = float(np.float32(1.0 / (SQT2 * SQT2)))  # tan^2(67.5)


def _gauss5():
    ax = np.arange(5, dtype=np.float32) - 2.0
    g1 = np.exp(-0.5 * ax * ax).astype(np.float32)
    return (g1 / g1.sum()).astype(np.float32)


def _vert_matrix(kind):
    g1 = _gauss5()
    I = np.eye(H, dtype=np.float64)
    X = np.pad(I, ((2, 2), (0, 0)), mode="reflect")
    B = np.zeros((H, H))
    for k in range(5):
        B += g1[k] * X[k:k + H]
    Y = np.pad(B, ((1, 1), (0, 0)), mode="edge")
    taps = [1.0, 2.0, 1.0] if kind == "smooth" else [-1.0, 0.0, 1.0]
    M = np.zeros((H, H))
    for k in range(3):
        if taps[k] != 0.0:
            M += taps[k] * Y[k:k + H]
    return M


def _build_consts():
    Ms = (_vert_matrix("smooth") * float(GRAY[2])).astype(np.float32)
    Md = (_vert_matrix("diff") * float(GRAY[2])).astype(np.float32)
    vs = np.zeros((P, NT, P), np.float32)
    vd = np.zeros((P, NT, P), np.float32)
    for t in range(NT):
        vs[:, t, :] = Ms[128 * t:128 * (t + 1), 128 * t:128 * (t + 1)].T
        vd[:, t, :] = Md[128 * t:128 * (t + 1), 128 * t:128 * (t + 1)].T
    vcor = np.zeros((36, 2, 18), np.float32)
    for b in range(3):
        in_rows = [128 * b + 122 + k for k in range(12)]
        out_rows = [128 * b + 125, 128 * b + 126, 128 * b + 127,
                    128 * (b + 1), 128 * (b + 1) + 1, 128 * (b + 1) + 2]
        for k, ir in enumerate(in_rows):
            for m, orr in enumerate(out_rows):
                vcor[12 * b + k, 0, 6 * b + m] = Ms[orr, ir]
                vcor[12 * b + k, 1, 6 * b + m] = Md[orr, ir]
    # fp16 row-shift matrices: up[k,m]=1 iff k=m+1 (U[m]=s[m+1]); dn: k=m-1
    shm = np.zeros((P, 2, P), np.float16)
    for m in range(P - 1):
        shm[m + 1, 0, m] = 1.0
    for m in range(1, P):
        shm[m - 1, 1, m] = 1.0
    return {"vs": vs, "vd": vd, "vcor": vcor, "shm16": shm}


_CACHE = {}


def _emit_image(nc, tc, pools, tens, img):
    """Generator: per-half stages (blocks [0,1] / [2,3]) so the caller can
    interleave 2 images x 2 halves = 4 pipeline streams."""
    import concourse.mybir as mybir
    AL = mybir.AluOpType
    AF = mybir.ActivationFunctionType
    F32 = mybir.dt.float32
    F16 = mybir.dt.float16
    U16 = mybir.dt.uint16

    pwork, pw16, psmall, (ppsumv, ppsumc) = pools
    (xdram, ydram, c_vs, c_vd, c_vcor, c_shm16, zrow16) = tens

    g1 = _gauss5()
    C0, C1, C2 = float(g1[2]), float(g1[1]), float(g1[0])
    R01 = float(np.float32(GRAY[0] / GRAY[1]))
    R12 = float(np.float32(GRAY[1] / GRAY[2]))

    HL = 2 * WP          # flat length of one half (1032)
    HV = HL - 4          # flat op length of one half (1028)

    def wt(tag):
        t = pwork.tile([P, NT, WP], F32, tag=tag, name=tag)
        return t, t[:].rearrange("p t w -> p (t w)")

    def wt16(tag, dt=F16):
        t = pw16.tile([P, NT, WP], dt, tag=tag, name=tag)
        return t, t[:].rearrange("p t w -> p (t w)")

    def hf(f, h, lo, ln):
        return f[:, h * HL + lo: h * HL + lo + ln]

    xa, xaf = wt("A")
    xb, xbf = wt("B")
    xc, xcf = wt("C")
    xv = xdram[img].rearrange("c (t p) w -> c p t w", p=P)

    # ---- S0: load input channels per half ----
    for h in range(2):
        sl = slice(2 * h, 2 * h + 2)
        for c, t in enumerate((xa, xb, xc)):
            nc.sync.dma_start(t[:, sl, INT], xv[c, :, sl, :])
        yield

    # ---- S1: grayscale (fp32; 1/GRAY[2] scale folded into vertical mats) --
    g, gf = xc, xcf          # gray lands in xc, base-2
    for h in range(2):
        nc.vector.scalar_tensor_tensor(hf(xbf, h, PAD, HL - PAD),
                                       hf(xaf, h, PAD, HL - PAD), R01,
                                       hf(xbf, h, PAD, HL - PAD),
                                       AL.mult, AL.add)
        nc.vector.scalar_tensor_tensor(hf(xcf, h, PAD, HL - PAD),
                                       hf(xbf, h, PAD, HL - PAD), R12,
                                       hf(xcf, h, PAD, HL - PAD),
                                       AL.mult, AL.add)
        sl = slice(2 * h, 2 * h + 2)
        nc.scalar.copy(g[:, sl, 1:2], g[:, sl, 3:4])
        nc.scalar.copy(g[:, sl, 0:1], g[:, sl, 4:5])
        nc.scalar.copy(g[:, sl, WP - 2:WP - 1], g[:, sl, WP - 4:WP - 3])
        nc.scalar.copy(g[:, sl, WP - 1:WP], g[:, sl, WP - 5:WP - 4])
        yield

    # ---- S2: horizontal 5-tap gaussian (fp32), gh base-2 = blur/C0 ----
    a1, a1f = xa, xaf
    a2, a2f = xb, xbf
    gh, ghf = wt("D")
    for h in range(2):
        nc.gpsimd.tensor_tensor(hf(a1f, h, 0, HV), hf(gf, h, 1, HV),
                                hf(gf, h, 3, HV), AL.add)
        nc.gpsimd.tensor_tensor(hf(a2f, h, 0, HV), hf(gf, h, 0, HV),
                                hf(gf, h, 4, HV), AL.add)
        nc.vector.scalar_tensor_tensor(hf(a1f, h, 0, HV), hf(a2f, h, 0, HV),
                                       C2 / C1, hf(a1f, h, 0, HV),
                                       AL.mult, AL.add)
        nc.vector.scalar_tensor_tensor(hf(ghf, h, 2, HV), hf(a1f, h, 0, HV),
                                       C1 / C0, hf(gf, h, 2, HV),
                                       AL.mult, AL.add)
        yield

    # ---- S3: vertical composite on PE (fp32) + corner halo fix ----
    u1, u1f = xa, xaf
    u2, u2f = xb, xbf
    cs = psmall.tile([36, W], F32, tag="cs", name="cs")
    cos = []
    for h in range(2):
        if h == 1:
            # corner chain (spans halves; bot stream owns it)
            for b in range(3):
                nc.sync.dma_start(cs[12 * b:12 * b + 6, :],
                                  gh[122:128, b, INT])
                nc.sync.dma_start(cs[12 * b + 6:12 * b + 12, :],
                                  gh[0:6, b + 1, INT])
            for ci in (0, 1):
                cps = ppsumc.tile([18, W], F32, tag="cps", name="cps")
                nc.tensor.matmul(cps[:], c_vcor[:, ci, :], cs[:],
                                 start=True, stop=True)
                co = psmall.tile([18, W], F32, tag="co", name="co")
                nc.scalar.copy(co[:], cps[:])
                cos.append(co)
        for (cm, u) in ((c_vs, u1), (c_vd, u2)):
            ps = ppsumv.tile([P, 2, W], F32, tag="psv", name="psv")
            for k in range(2):
                t = 2 * h + k
                nc.tensor.matmul(ps[:, k, :], cm[:, t, :], gh[:, t, INT],
                                 start=True, stop=True)
            nc.scalar.copy(u[:, 2 * h:2 * h + 2, INT], ps[:])
        if h == 1:
            for ci, u in ((0, u1), (1, u2)):
                co = cos[ci]
                for b in range(3):
                    nc.sync.dma_start(u[125:128, b, INT],
                                      co[6 * b:6 * b + 3, :])
                    nc.sync.dma_start(u[0:3, b + 1, INT],
                                      co[6 * b + 3:6 * b + 6, :])
        sl = slice(2 * h, 2 * h + 2)
        for u in (u1, u2):
            nc.scalar.copy(u[:, sl, 1:2], u[:, sl, 2:3])
            nc.scalar.copy(u[:, sl, WP - 2:WP - 1], u[:, sl, WP - 3:WP - 2])
        yield

    # ---- S4: 3-tap horizontal gradients (fp32, base-0) ----
    gx, gxf = gh, ghf        # NOTE: gh still needed by bot's corner path;
    # halves write into gh's own storage only after corner consumed it: the
    # cs loads read gh rows and complete before h3tap's in-place overwrite
    # (tile dep tracking orders DMA-read-then-write on overlap).
    ay, ayf = wt("E")
    gy, gyf = ay, ayf
    for h in range(2):
        nc.vector.tensor_tensor(hf(gxf, h, 0, HV), hf(u1f, h, 3, HV),
                                hf(u1f, h, 1, HV), AL.subtract)
        nc.gpsimd.tensor_tensor(hf(ayf, h, 0, HV), hf(u2f, h, 1, HV),
                                hf(u2f, h, 3, HV), AL.add)
        nc.vector.scalar_tensor_tensor(hf(ayf, h, 0, HV), hf(u2f, h, 2, HV),
                                       2.0, hf(ayf, h, 0, HV),
                                       AL.mult, AL.add)
        yield

    # ---- S5: squares (ACT fp32) + s16 fp16 field + masks ----
    sx, sxf = u1, u1f
    sy, syf = u2, u2f
    s16, s16f = wt16("S16")
    notch, notchf = wt16("NCH", U16)
    cvm, cvmf = wt16("CV", U16)
    pxy, pxyf = wt16("PXY")
    md1, md1f = wt16("MD", U16)
    for h in range(2):
        sl = slice(2 * h, 2 * h + 2)
        nc.scalar.activation(hf(sxf, h, 0, HV), hf(gxf, h, 0, HV),
                             AF.Square, 0.0, C0)
        nc.scalar.activation(hf(syf, h, 0, HV), hf(gyf, h, 0, HV),
                             AF.Square, 0.0, C0)
        nc.vector.tensor_tensor(hf(s16f, h, 2, HV), hf(sxf, h, 0, HV),
                                hf(syf, h, 0, HV), AL.add)
        nc.gpsimd.memset(s16[:, sl, 0:PAD], 0.0)
        nc.gpsimd.memset(s16[:, sl, WP - PAD:WP], 0.0)
        nc.vector.scalar_tensor_tensor(hf(notchf, h, 0, HV), hf(sxf, h, 0, HV),
                                       T2, hf(syf, h, 0, HV),
                                       AL.mult, AL.is_lt)
        nc.vector.scalar_tensor_tensor(hf(cvmf, h, 0, HV), hf(sxf, h, 0, HV),
                                       TH2, hf(syf, h, 0, HV),
                                       AL.mult, AL.is_lt)
        nc.vector.tensor_tensor(hf(pxyf, h, 0, HV), hf(gxf, h, 0, HV),
                                hf(gyf, h, 0, HV), AL.mult)
        nc.vector.tensor_scalar(hf(md1f, h, 0, HV), hf(pxyf, h, 0, HV),
                                0.0, None, AL.is_gt)
        yield

    # ---- S6: row-shifted planes U16[r]=s16[r+1], D16[r]=s16[r-1] (PE) ----
    U16t, U16f = wt16("U16")
    D16t, D16f = wt16("D16")
    for h in range(2):
        sl = slice(2 * h, 2 * h + 2)
        for (ci, pl) in ((0, U16t), (1, D16t)):
            ps = ppsumv.tile([P, 2, W], F32, tag="psv", name="psv")
            for k in range(2):
                t = 2 * h + k
                nc.tensor.matmul(ps[:, k, :], c_shm16[:, ci, :],
                                 s16[:, t, INT], start=True, stop=True)
            nc.scalar.copy(pl[:, sl, INT], ps[:])
        for pl in (U16t, D16t):
            nc.gpsimd.memset(pl[:, sl, 0:PAD], 0.0)
            nc.gpsimd.memset(pl[:, sl, WP - PAD:WP], 0.0)
        if h == 1:
            nc.sync.dma_start(U16t[127:128, 0:NT - 1, INT],
                              s16[0:1, 1:NT, INT])
            nc.sync.dma_start(U16t[127:128, NT - 1, INT], zrow16[0:1, :])
            nc.sync.dma_start(D16t[0:1, 1:NT, INT],
                              s16[127:128, 0:NT - 1, INT])
            nc.gpsimd.memset(D16t[0:1, 0, INT], 0.0)
        yield

    # ---- S7: pair maxes + select chain (fp16) ----
    selx, selxf = wt16("SELX")
    m1t, m1tf = wt16("M1")
    mvt, mvtf = wt16("MV")
    sel, self_ = wt16("SEL")
    for h in range(2):
        nc.vector.tensor_tensor(hf(selxf, h, 0, HV), hf(U16f, h, 1, HV),
                                hf(D16f, h, 3, HV), AL.max)
        nc.vector.tensor_tensor(hf(m1tf, h, 0, HV), hf(U16f, h, 3, HV),
                                hf(D16f, h, 1, HV), AL.max)
        nc.vector.tensor_tensor(hf(mvtf, h, 0, HV), hf(U16f, h, 2, HV),
                                hf(D16f, h, 2, HV), AL.max)
        nc.vector.tensor_tensor(hf(self_, h, 0, HV), hf(s16f, h, 1, HV),
                                hf(s16f, h, 3, HV), AL.max)
        nc.vector.copy_predicated(hf(selxf, h, 0, HV), hf(md1f, h, 0, HV),
                                  hf(m1tf, h, 0, HV))
        nc.vector.copy_predicated(hf(selxf, h, 0, HV), hf(cvmf, h, 0, HV),
                                  hf(mvtf, h, 0, HV))
        nc.vector.copy_predicated(hf(self_, h, 0, HV), hf(notchf, h, 0, HV),
                                  hf(selxf, h, 0, HV))
        yield

    # ---- S8: keep, magnitude, clip, store (fp16 out) ----
    keep, keepf = m1t, m1tf
    mag, magf = mvt, mvtf
    out16, out16f = selx, selxf
    yv = ydram[img].rearrange("(t p) w -> p t w", p=P)
    for h in range(2):
        nc.vector.tensor_tensor(hf(keepf, h, 0, HV), hf(s16f, h, 2, HV),
                                hf(self_, h, 0, HV), AL.is_gt)
        nc.scalar.activation(hf(magf, h, 0, HV), hf(s16f, h, 2, HV),
                             AF.Sqrt, 0.0, 1.0)
        nc.vector.tensor_scalar(hf(magf, h, 0, HV), hf(magf, h, 0, HV),
                                1.0, None, AL.min)
        nc.vector.tensor_tensor(hf(out16f, h, 0, HV), hf(magf, h, 0, HV),
                                hf(keepf, h, 0, HV), AL.mult)
        sl = slice(2 * h, 2 * h + 2)
        nc.sync.dma_start(yv[:, sl, :], out16[:, sl, 0:W])
        yield


def _build():
    import concourse.bacc as bacc
    import concourse.mybir as mybir
    from concourse import tile
    F32 = mybir.dt.float32
    F16 = mybir.dt.float16

    nc = bacc.Bacc("TRN2", target_bir_lowering=False, debug=False,
                   num_devices=NCORES)
    xdram = nc.declare_dram_parameter("xc", [NI, 3, H, W], F32, isOutput=False)
    c_vs_d = nc.declare_dram_parameter("vs", [P, NT, P], F32, isOutput=False)
    c_vd_d = nc.declare_dram_parameter("vd", [P, NT, P], F32, isOutput=False)
    c_vcor_d = nc.declare_dram_parameter("vcor", [36, 2, 18], F32,
                                         isOutput=False)
    c_shm_d = nc.declare_dram_parameter("shm16", [P, 2, P], F16,
                                        isOutput=False)
    ydram = nc.declare_dram_parameter("y", [NI, H, W], F16, isOutput=True)

    with tile.TileContext(nc) as tc:
        with tc.tile_pool(name="pconst", bufs=1) as pconst, \
             tc.tile_pool(name="pwork", bufs=2) as pwork, \
             tc.tile_pool(name="pw16", bufs=2) as pw16, \
             tc.tile_pool(name="psmall", bufs=2) as psmall, \
             tc.tile_pool(name="ppsumv", bufs=3, space="PSUM") as ppsumv, \
             tc.tile_pool(name="ppsumc", bufs=2, space="PSUM") as ppsumc:
            c_vs = pconst.tile([P, NT, P], F32, tag="cvs")
            nc.sync.dma_start(c_vs[:], c_vs_d[:])
            c_vd = pconst.tile([P, NT, P], F32, tag="cvd")
            nc.sync.dma_start(c_vd[:], c_vd_d[:])
            c_vcor = pconst.tile([36, 2, 18], F32, tag="cvcor")
            nc.sync.dma_start(c_vcor[:], c_vcor_d[:])
            c_shm16 = pconst.tile([P, 2, P], F16, tag="cshm")
            nc.sync.dma_start(c_shm16[:], c_shm_d[:])
            zrow16 = pconst.tile([1, W], F16, tag="zr16")
            nc.gpsimd.memset(zrow16[:], 0.0)

            pools = (pwork, pw16, psmall, (ppsumv, ppsumc))
            tens = (xdram, ydram, c_vs, c_vd, c_vcor, c_shm16, zrow16)
            import os
            nrep = int(os.environ.get("KREPEAT", "1"))
            for rep in range(nrep):
                gens = [_emit_image(nc, tc, pools, tens, img)
                        for img in range(NI)]
                done = [False] * NI
                while not all(done):
                    for i, gi in enumerate(gens):
                        if not done[i]:
                            try:
                                next(gi)
                            except StopIteration:
                                done[i] = True

    nc.compile()
    return nc


def _get_nc():
    if "nc" not in _CACHE:
        _CACHE["nc"] = _build()
        _CACHE["consts"] = _build_consts()
    return _CACHE["nc"], _CACHE["consts"]


def kernel(x):
    from concourse.bass_utils import run_bass_kernel_spmd
    x = np.ascontiguousarray(np.asarray(x), dtype=np.float32)
    assert x.shape == (16, 3, H, W), x.shape
    nc, consts = _get_nc()
    in_maps = []
    for c in range(NCORES):
        m = {"xc": x[NI * c:NI * (c + 1)]}
        m.update(consts)
        in_maps.append(m)
    res = run_bass_kernel_spmd(nc, in_maps, list(range(NCORES)))
    y = np.concatenate([res.results[c]["y"] for c in range(NCORES)], axis=0)
    y = y.astype(np.float32)[:, None]          # widen fp16 -> f32, add C dim
    return np.repeat(y, 3, axis=1)             # replicate 3 identical channels


if __name__ == "__main__":
    import golden
    rng = np.random.default_rng(0)
    x = rng.random((16, 3, H, W), dtype=np.float32)
    y = kernel(x)
    ref = golden.reference_np(x)
    d = y - ref
    print("L2 rel:", np.linalg.norm(d) / np.linalg.norm(ref))
    print("absmax:", np.abs(d).max(), " bigpix:", (np.abs(d) > 1e-3).sum())
